# revision 13
# baseline (speedup 1.0000x reference)
"""Trainium2 Bass kernel for LocalSpatialSimilarity.

Per sample (B=16, C=256, H=W=64, N=4096 pixels):
  s[p]  = sum_c x[c,p]                (channel sum)
  q[p]  = sum_c x[c,p]^2              (channel sum of squares)
  box   = 3x3 zero-padded box-sum of s (reshaped to 64x64)
  sim   = (box/9 * s) / sqrt(max(q * box^2 * 256/81, 1e-12))
  out   = softmax over p of (mask ? -inf : -sim)
        = (mask ? 0 : exp(-sim)) / total        (sim bounded in [-1,1] -> no
                                                 max-subtraction needed)

Sharding: pure data parallel, 2 samples per core across 8 cores.

On-chip layout: channels on partitions (two 128-chunks), pixels on the free
dim.  Channel reductions are ones-matmuls on the tensor engine into a
[8, 512] PSUM tile (stationary is an indicator column so block j of 512
pixels lands on psum partition j).  Spatial phase runs on a [64 rows,
2 samples, 64 cols] layout where the 3x3 box filter is partition-shifted /
free-shifted adds against a zero-padded tile.
"""

import sys

sys.path.insert(0, "/opt/trn_rl_repo")

import numpy as np

import concourse.bacc as bacc
import concourse.mybir as mybir
import concourse.tile as tile
from concourse.bass_utils import run_bass_kernel_spmd

B, C, H, W = 16, 256, 64, 64
N = H * W
NCORES = 8
SPC = B // NCORES  # samples per core
EPS2 = 1e-12
FP32 = mybir.dt.float32

# float32r: relaxed-precision fp32 matmul, 4x tensor-engine throughput.
MM_DT = mybir.dt.float32r

AF = mybir.ActivationFunctionType
ALU = mybir.AluOpType


def _kernel_body(ctx, tc, x, mask, vband, out, mm_dt=MM_DT):
    nc = tc.nc
    HB = 2048  # pixels per spatial half

    consts = ctx.enter_context(tc.tile_pool(name="consts", bufs=1))
    xp = ctx.enter_context(tc.tile_pool(name="xp", bufs=4))
    sqp = ctx.enter_context(tc.tile_pool(name="sqp", bufs=3))
    rows = ctx.enter_context(tc.tile_pool(name="rows", bufs=4))
    single = ctx.enter_context(tc.tile_pool(name="single", bufs=1))
    psa = ctx.enter_context(tc.tile_pool(name="psa", bufs=4, space="PSUM"))
    pss = ctx.enter_context(tc.tile_pool(name="pss", bufs=1, space="PSUM"))

    # Stationary band: D[k, c] = 1 iff c == 7.  Slice [:, 7-j:15-j] is a
    # [128, 8] matrix whose only nonzero column is j, so the ones-matmul
    # lands block j's column sums on psum partition j (zeros elsewhere,
    # accumulated away).
    band = consts.tile([128, 15], FP32)
    nc.vector.memset(band[:], 0.0)
    nc.vector.memset(band[:, 7:8], 1.0)
    ones = consts.tile([128, 64], FP32)
    nc.vector.memset(ones[:], 1.0)
    # Tridiagonal 64x64 ones-band (host-provided): vertical 3-tap box sum as
    # a partition-space matmul (SBUF APs cannot start at unaligned
    # partitions, so partition-shifted adds are not expressible).
    band64 = consts.tile([64, 64], FP32)
    nc.sync.dma_start(out=band64[:], in_=vband.ap())

    # Pair-batched spatial tiles: [row r, sample s, col c].
    Sb = single.tile([64, SPC, 64], FP32)
    Qt = single.tile([64, SPC, 64], FP32)

    # Mask, cast bool->f32 during DMA, then scaled to +1e30 ("-inf" additive).
    maskf = single.tile([64, SPC, 64], FP32)
    nc.gpsimd.dma_start(out=maskf[:], in_=mask.ap().rearrange("s (r c) -> r s c", c=64))
    mb = single.tile([64, SPC, 64], FP32)
    nc.vector.tensor_scalar_mul(mb[:], maskf[:], 1e30)

    for s in range(SPC):
        ps_s = psa.tile([8, 512], FP32, tag="acc")
        ps_q = psa.tile([8, 512], FP32, tag="acc")
        for h in range(2):
            x0 = xp.tile([128, HB], FP32, tag="x")
            nc.sync.dma_start(out=x0[:], in_=x[s, 0:128, HB * h : HB * (h + 1)])
            x1 = xp.tile([128, HB], FP32, tag="x")
            nc.sync.dma_start(out=x1[:], in_=x[s, 128:256, HB * h : HB * (h + 1)])
            # Fold the two channel chunks before the matmul: halves PE work.
            sf = sqp.tile([128, HB], FP32, tag="sf")
            nc.vector.tensor_add(sf[:], x0[:], x1[:])
            sq0 = sqp.tile([128, HB], FP32, tag="sq0")
            nc.scalar.activation(sq0[:], x0[:], AF.Square)
            qf = sqp.tile([128, HB], FP32, tag="qf")
            nc.vector.tensor_mul(qf[:], x1[:], x1[:])
            nc.vector.tensor_add(qf[:], qf[:], sq0[:])
            for j4 in range(4):
                j = 4 * h + j4
                st = band[:, 7 - j : 15 - j]
                nc.tensor.matmul(
                    ps_s[:],
                    st,
                    sf[:, 512 * j4 : 512 * (j4 + 1)],
                    start=h == 0 and j4 == 0,
                    stop=h == 1 and j4 == 3,
                )
                nc.tensor.matmul(
                    ps_q[:],
                    st,
                    qf[:, 512 * j4 : 512 * (j4 + 1)],
                    start=h == 0 and j4 == 0,
                    stop=h == 1 and j4 == 3,
                )
        s_sb = rows.tile([8, 512], FP32, tag="srow")
        q_sb = rows.tile([8, 512], FP32, tag="qrow")
        nc.scalar.copy(s_sb[:], ps_s[:])
        nc.scalar.copy(q_sb[:], ps_q[:])
        # Reshape [8, 512] -> [64, 64]: both APs enumerate pixels in order.
        nc.sync.dma_start(out=Sb[:, s, :], in_=s_sb[:])
        nc.sync.dma_start(out=Qt[:, s, :], in_=q_sb[:])

    # 3x3 box-sum of S with zero padding: vertical 3-tap via tridiagonal
    # matmul over the row-partition dim, horizontal via free-shifted adds.
    v_ps = pss.tile([64, SPC * 64], FP32, tag="vps")
    nc.tensor.matmul(
        v_ps[:], band64[:], Sb[:].rearrange("r s c -> r (s c)"), start=True, stop=True
    )
    Hb = single.tile([64, SPC, 66], FP32)  # cols 0 and 65 stay zero
    nc.vector.memset(Hb[:], 0.0)
    nc.scalar.copy(Hb[:, :, 1:65], v_ps[:].rearrange("r (s c) -> r s c", c=64))
    T1 = single.tile([64, SPC, 64], FP32)
    nc.vector.tensor_add(T1[:], Hb[:, :, 0:64], Hb[:, :, 1:65])
    BOX = single.tile([64, SPC, 64], FP32)
    nc.vector.tensor_add(BOX[:], T1[:], Hb[:, :, 2:66])

    # D = max(box^2 * q * 256/81, eps^2);  R = D^-1/2 via exp(-0.5 ln D)
    # (Rsqrt activation is disallowed for accuracy reasons).
    P = single.tile([64, SPC, 64], FP32)
    nc.vector.tensor_mul(P[:], BOX[:], BOX[:])
    P2 = single.tile([64, SPC, 64], FP32)
    nc.vector.tensor_mul(P2[:], P[:], Qt[:])
    Dt = single.tile([64, SPC, 64], FP32)
    nc.vector.tensor_scalar(
        Dt[:], P2[:], 256.0 / 81.0, EPS2, op0=ALU.mult, op1=ALU.max
    )
    L = single.tile([64, SPC, 64], FP32)
    nc.scalar.activation(L[:], Dt[:], AF.Ln)
    R = single.tile([64, SPC, 64], FP32)
    nc.scalar.activation(R[:], L[:], AF.Exp, scale=-0.5)

    # U = box * s * R;  exp(-(U + 1e30*mask)/9) = masked exp(-sim)
    T = single.tile([64, SPC, 64], FP32)
    nc.vector.tensor_mul(T[:], BOX[:], Sb[:])
    U = single.tile([64, SPC, 64], FP32)
    nc.vector.tensor_mul(U[:], T[:], R[:])
    U2 = single.tile([64, SPC, 64], FP32)
    nc.vector.tensor_add(U2[:], U[:], mb[:])
    EM = single.tile([64, SPC, 64], FP32)
    rowsum = single.tile([64, SPC], FP32)
    for s in range(SPC):
        nc.scalar.activation(
            EM[:, s, :],
            U2[:, s, :],
            AF.Exp,
            scale=-1.0 / 9.0,
            accum_out=rowsum[:, s : s + 1],
        )

    # Per-sample totals: 64->1 ones-matmul, broadcast back 1->64, reciprocal.
    tot_ps = pss.tile([1, SPC], FP32, tag="tot")
    nc.tensor.matmul(tot_ps[:], ones[0:64, 0:1], rowsum[:], start=True, stop=True)
    tots = single.tile([1, SPC], FP32)
    nc.scalar.copy(tots[:], tot_ps[:])
    totb_ps = pss.tile([64, SPC], FP32, tag="totb")
    nc.tensor.matmul(totb_ps[:], ones[0:1, 0:64], tots[:], start=True, stop=True)
    rec = single.tile([64, SPC], FP32)
    nc.vector.reciprocal(rec[:], totb_ps[:])

    OUTt = single.tile([64, SPC, 64], FP32)
    for s in range(SPC):
        nc.vector.tensor_scalar_mul(
            OUTt[:, s, :], EM[:, s, :], rec[:, s : s + 1]
        )
    nc.sync.dma_start(
        out=out.ap().rearrange("s (r c) -> r s c", c=64), in_=OUTt[:]
    )


_NC_CACHE = {}


def _build(mm_dt=MM_DT):
    key = str(mm_dt)
    if key in _NC_CACHE:
        return _NC_CACHE[key]
    nc = bacc.Bacc("TRN2", target_bir_lowering=False, debug=False)
    x = nc.declare_dram_parameter("x", [SPC, C, N], FP32, isOutput=False)
    mask = nc.declare_dram_parameter("mask", [SPC, N], mybir.dt.uint8, isOutput=False)
    vband = nc.declare_dram_parameter("vband", [64, 64], FP32, isOutput=False)
    out = nc.declare_dram_parameter("out", [SPC, N], FP32, isOutput=True)
    from contextlib import ExitStack

    with tile.TileContext(nc) as tc, ExitStack() as ctx:
        _kernel_body(ctx, tc, x, mask, vband, out, mm_dt)
    nc.compile()
    _NC_CACHE[key] = nc
    return nc


def band_matrix() -> np.ndarray:
    idx = np.arange(64)
    return (np.abs(idx[:, None] - idx[None, :]) <= 1).astype(np.float32)


def kernel(x: np.ndarray, prev_drop_mask: np.ndarray) -> np.ndarray:
    nc = _build()
    xs = np.ascontiguousarray(np.asarray(x), dtype=np.float32).reshape(B, C, N)
    ms = np.asarray(prev_drop_mask).astype(np.uint8).reshape(B, N)
    vb = band_matrix()
    in_maps = [
        {
            "x": xs[i * SPC : (i + 1) * SPC],
            "mask": ms[i * SPC : (i + 1) * SPC],
            "vband": vb,
        }
        for i in range(NCORES)
    ]
    res = run_bass_kernel_spmd(nc, in_maps, list(range(NCORES)))
    outs = [res.results[i]["out"] for i in range(NCORES)]
    return np.concatenate(outs, axis=0).reshape(B, H, W)
